# revision 8
# baseline (speedup 1.0000x reference)
"""Trainium2 Bass kernel for nn_AttGraphConvLayer (v2, transposed-E).

Reference computation (per batch b):
    z   = nodes @ w                          [N, D]
    att = leaky_relu(concat(z1, z2) @ attention)      per edge
    scores = (Cmat^T * att^T) @ Nmat         [N, N]
    adj    = Cmat^T @ Nmat                   [N, N]
    logits = scores + (1 - adj) * (-1e9)
    out = leaky_relu(softmax(logits, -1) * adj @ z)   [N, D]

Identities (Cmat/Nmat one-hot):
  * att_e = leaky(u[src] + v[dst]) with u = z @ a_top, v = z @ a_bot.
  * For rows whose max edge multiplicity is 1 (all but ~18/core), the
    reference reduces EXACTLY to out[n] = leaky(sum_m A[n,m] z[m] / Z_n)
    with A = adj .* exp(leaky(u[n]+v[m])), Z_n = sum_m A[n,m]; the exp
    row-max shift cancels between numerator and denominator in fp32.
  * Rows with multiplicity >= 2 collapse (reference fp32 artifact) to
    equal weights over the max-multiplicity edges -> host-computed.
    Rows with no edges are exactly 0 -> host-set (device yields NaN
    there via 1/Z=inf, overwritten).

v2 layout trick: the attention matrix is produced TRANSPOSED,
  ET[m, n] = adjT[m, n] * exp(leaky(v[m] + u[n]))
(partition = m = source-of-message axis, free = n = own row axis), which
is directly the lhsT of the output matmul out = ET^T @ z. This removes
the DMA transposes, turns the mask multiply into a bf16 2x-mode DVE op,
and lets output matmuls consume ET chunk-by-chunk as ACT produces them.
Z row sums come from PE ones-matmuls into a [1, H] psum accumulated
alongside; rcpf = 1/Z is transposed back via 4 tiny PE transposes.

Sharding: 8 cores = 4 batches x 2 row-halves (partition by source node).
The host rotates the node axis per core so its 512 output rows are the
first 512 node columns.
"""

import sys

for _p in ("/opt/trn_rl_repo", "/root/.axon_site/_ro/trn_rl_repo"):
    if _p not in sys.path:
        sys.path.insert(0, _p)

import numpy as np

B, E, N, F, D = 4, 8192, 1024, 512, 512
H = N // 2          # rows per core
P = 128
ALPHA = 0.2
N_CORES = 8
NC_F = F // P       # 4 feature chunks
NC_N = N // P       # 8 node (m) chunks
NC_H = H // P       # 4 own-row chunks

_compiled = None


def _build():
    import concourse.bacc as bacc
    import concourse.tile as tile
    import concourse.mybir as mybir

    dt = mybir.dt
    f32 = dt.float32
    bf16 = dt.bfloat16
    fp8 = dt.float8e4
    Act = mybir.ActivationFunctionType

    nc = bacc.Bacc("TRN2", target_bir_lowering=False, debug=False,
                   num_devices=N_CORES)

    # dram tensors are laid out host-side exactly as SBUF wants them
    nT = nc.dram_tensor("nT", [P, NC_F, N], bf16, kind="ExternalInput").ap()
    wsb = nc.dram_tensor("wsb", [P, NC_F, D], bf16, kind="ExternalInput").ap()
    wbc = nc.dram_tensor("wbc", [P, NC_F, 2], bf16, kind="ExternalInput").ap()
    adjT = nc.dram_tensor("adjT", [P, NC_N, H], fp8, kind="ExternalInput").ap()
    out = nc.dram_tensor("out", [P, NC_H, D], bf16, kind="ExternalOutput").ap()

    with tile.TileContext(nc) as tc:
        with tc.tile_pool(name="singles", bufs=1) as singles:
            nT_sb = singles.tile([P, NC_F, N], bf16, name="nT_sb")
            w_sb = singles.tile([P, NC_F, D], bf16, name="w_sb")
            wb_sb = singles.tile([P, NC_F, 2], bf16, name="wb_sb")
            adjT_sb = singles.tile([P, NC_N, H], bf16, name="adjT_sb")
            z_sb = singles.tile([P, NC_N, D], bf16, name="z_sb")
            ET_sb = singles.tile([P, NC_N, H], bf16, name="ET_sb")
            U_bc = singles.tile([P, H], f32, name="U_bc")
            uv_sb = singles.tile([2, H], f32, name="uv_sb")
            vb_sb = singles.tile([2, H], f32, name="vb_sb")
            u_row = singles.tile([1, H], f32, name="u_row")
            v_col = singles.tile([P, NC_N], f32, name="v_col")
            ones_c = singles.tile([P, 1], bf16, name="ones_c")
            ones_r = singles.tile([1, P], f32, name="ones_r")
            ident1 = singles.tile([1, 1], f32, name="ident1")
            dumm = singles.tile([1, 1], f32, name="dumm")
            Z_sb = singles.tile([1, H], f32, name="Z_sb")
            Zr_sb = singles.tile([1, H], f32, name="Zr_sb")
            rcpf = singles.tile([P, NC_H], f32, name="rcpf")

            nc.vector.memset(ones_c, 1.0)
            nc.gpsimd.memset(ones_r, 1.0)
            nc.gpsimd.memset(ident1, 1.0)
            # preload the exp ACT table during the DMA phase
            nc.scalar.activation(dumm, ident1, Act.Exp)

            # ---- input DMAs ----
            # tiny first; then nT column-halves sequential on the sync ring
            # (matvec + z chunks 0-3 unlock after the first half) with w in
            # parallel on the scalar ring; adjT fp8->bf16 cast-loads on the
            # gpsimd SWDGE ring, one per m-chunk so ET mults unlock early.
            nc.scalar.dma_start(out=wb_sb, in_=wbc)
            nc.sync.dma_start(out=nT_sb[:, :, 0:H], in_=nT[:, :, 0:H])
            nc.scalar.dma_start(out=w_sb, in_=wsb)
            nc.sync.dma_start(out=nT_sb[:, :, H:N], in_=nT[:, :, H:N])
            for cm in range(NC_N):
                nc.gpsimd.dma_start(out=adjT_sb[:, cm, :], in_=adjT[:, cm, :])

            with tc.tile_pool(name="vu_ps", bufs=1, space="PSUM") as vu_ps, \
                 tc.tile_pool(name="usc", bufs=1, space="PSUM") as usc:
                # u/v matvecs: three M=1 accumulations so every row the PE
                # later reads sits on partition 0 (base-partition rule)
                v0p = vu_ps.tile([1, H], f32, name="v0p", tag="v0p")
                v1p = vu_ps.tile([1, H], f32, name="v1p", tag="v1p")
                u_p = vu_ps.tile([1, H], f32, name="u_p", tag="u_p")
                for cf in range(NC_F):
                    nc.tensor.matmul(v0p, lhsT=wb_sb[:, cf, 0:1],
                                     rhs=nT_sb[:, cf, 0:H],
                                     start=(cf == 0), stop=(cf == NC_F - 1))
                for cf in range(NC_F):
                    nc.tensor.matmul(u_p, lhsT=wb_sb[:, cf, 1:2],
                                     rhs=nT_sb[:, cf, 0:H],
                                     start=(cf == 0), stop=(cf == NC_F - 1))
                nc.vector.tensor_copy(uv_sb[0:1, :], v0p)
                nc.vector.tensor_copy(u_row, u_p)
                for cf in range(NC_F):
                    nc.tensor.matmul(v1p, lhsT=wb_sb[:, cf, 0:1],
                                     rhs=nT_sb[:, cf, H:N],
                                     start=(cf == 0), stop=(cf == NC_F - 1))
                nc.vector.tensor_copy(vb_sb[0:1, :], v1p)

                # v as per-partition columns (8 tiny PE transposes), and
                # u broadcast along partitions (rank-1 ones matmul)
                vc_ps = usc.tile([P, NC_N], f32, name="vc_ps", tag="vc")
                for q in range(NC_H):
                    nc.tensor.transpose(vc_ps[:, q:q + 1],
                                        uv_sb[0:1, q * P:(q + 1) * P], ident1)
                nc.vector.tensor_copy(v_col[:, 0:NC_H], vc_ps[:, 0:NC_H])
                U_ps = usc.tile([P, H], f32, name="U_ps", tag="ups")
                nc.tensor.matmul(U_ps, lhsT=ones_r, rhs=u_row,
                                 start=True, stop=True)
                nc.vector.tensor_copy(U_bc, U_ps)
                for q in range(NC_H, NC_N):
                    nc.tensor.transpose(
                        vc_ps[:, q:q + 1],
                        vb_sb[0:1, (q - NC_H) * P:(q - NC_H + 1) * P], ident1)
                nc.vector.tensor_copy(v_col[:, NC_H:NC_N], vc_ps[:, NC_H:NC_N])

            # ---- z = nodes @ w, chunk cn completes asap; cast to bf16 ----
            o_ps = tc.alloc_tile_pool(name="o_ps", bufs=1, space="PSUM")
            ops = [o_ps.tile([P, D], f32, name=f"op_{r}", tag=f"op_{r}")
                   for r in range(NC_H)]
            Z_ps = o_ps.tile([1, H], f32, name="Z_ps", tag="zps")
            z_ps = tc.alloc_tile_pool(name="z_ps", bufs=3, space="PSUM")
            for cn in range(NC_N):
                zp = z_ps.tile([P, D], f32, name=f"zp_{cn}", tag="zp")
                for cf in range(NC_F):
                    nc.tensor.matmul(zp,
                                     lhsT=nT_sb[:, cf, cn * P:(cn + 1) * P],
                                     rhs=w_sb[:, cf, :],
                                     start=(cf == 0), stop=(cf == NC_F - 1))
                nc.vector.tensor_copy(z_sb[:, cn, :], zp)

            # ---- per m-chunk: ET = adjT .* exp(leaky(v + u)), then the
            # output matmuls consume each chunk as it lands ----
            with tc.tile_pool(name="pscr", bufs=3) as pscr, \
                 tc.tile_pool(name="escr", bufs=3) as escr:
                for cm in range(NC_N):
                    pt = pscr.tile([P, H], f32, name=f"pt_{cm}", tag="pt")
                    nc.scalar.activation(pt, U_bc, Act.Prelu,
                                         bias=v_col[:, cm:cm + 1], scale=1.0,
                                         alpha=ALPHA)
                    et = escr.tile([P, H], bf16, name=f"et_{cm}", tag="et")
                    nc.scalar.activation(et, pt, Act.Exp)
                    nc.vector.tensor_mul(ET_sb[:, cm, :], et,
                                         adjT_sb[:, cm, :])
                    for r in range(NC_H):
                        nc.tensor.matmul(
                            ops[r], lhsT=ET_sb[:, cm, r * P:(r + 1) * P],
                            rhs=z_sb[:, cm, :],
                            start=(cm == 0), stop=(cm == NC_N - 1))
                    nc.tensor.matmul(Z_ps, lhsT=ones_c, rhs=ET_sb[:, cm, :],
                                     start=(cm == 0), stop=(cm == NC_N - 1))
            z_ps.release()

            # ---- rcpf = 1/Z as per-partition columns ----
            rc_ps = tc.alloc_tile_pool(name="rc_ps", bufs=1, space="PSUM")
            nc.vector.tensor_copy(Z_sb, Z_ps)
            nc.vector.reciprocal(Zr_sb, Z_sb)
            rcp_t = rc_ps.tile([P, NC_H], f32, name="rcp_t", tag="rct")
            for q in range(NC_H):
                nc.tensor.transpose(rcp_t[:, q:q + 1],
                                    Zr_sb[:, q * P:(q + 1) * P], ident1)
            nc.vector.tensor_copy(rcpf, rcp_t)

            # ---- out = leaky(rcpf * psum), store ----
            with tc.tile_pool(name="oscr", bufs=2) as oscr:
                for r in range(NC_H):
                    o_l = oscr.tile([P, D], bf16, name=f"ol_{r}", tag="ol")
                    nc.scalar.activation(o_l, ops[r], Act.Prelu, bias=0.0,
                                         scale=rcpf[:, r:r + 1], alpha=ALPHA)
                    eng = (nc.gpsimd, nc.sync, nc.scalar, nc.gpsimd)[r]
                    eng.dma_start(out=out[:, r, :], in_=o_l)
            rc_ps.release()
            o_ps.release()

    nc.compile()
    return nc


def _get_compiled():
    global _compiled
    if _compiled is None:
        _compiled = _build()
    return _compiled


def _in_maps(nodes, Cmat, Nmat, w, attention):
    import ml_dtypes
    f8 = ml_dtypes.float8_e4m3
    bf = ml_dtypes.bfloat16
    nodes = np.asarray(nodes, dtype=np.float32)
    w = np.ascontiguousarray(np.asarray(w, dtype=np.float32))
    attention = np.asarray(attention, dtype=np.float32)
    fixups = []
    wa = w @ attention[:D, 0]                     # [F] = w @ a_top
    wb = w @ attention[D:, 0]                     # [F] = w @ a_bot
    wb_dev = np.ascontiguousarray(
        np.stack([wb.reshape(NC_F, P).T, wa.reshape(NC_F, P).T],
                 axis=2).astype(bf))              # [P, NC_F, 2]: col0=v, col1=u
    w_dev = np.ascontiguousarray(
        w.reshape(NC_F, P, D).transpose(1, 0, 2).astype(bf))
    maps = []
    for core in range(N_CORES):
        b, h = divmod(core, 2)
        src = Cmat[b].argmax(axis=1)
        dst = Nmat[b].argmax(axis=1)
        srcp = (src - h * H) % N                  # rotated node ids
        dstp = (dst - h * H) % N
        own = srcp < H
        sp, dp = srcp[own], dstp[own]
        # transposed adjacency counts: adjT[m, n] = #edges (src=n, dst=m)
        adjT = np.zeros((N, H), np.float32)
        np.add.at(adjT, (dp, sp), 1.0)
        adjT_dev = np.ascontiguousarray(
            adjT.reshape(NC_N, P, H).transpose(1, 0, 2).astype(f8))
        # per-own-row max edge multiplicity (host stat for fixups)
        key = sp.astype(np.int64) * N + dp
        uq, c = np.unique(key, return_counts=True)
        amax = np.zeros(H, np.float32)
        np.maximum.at(amax, (uq // N).astype(np.int64), c.astype(np.float32))
        duprows = []
        for n in np.nonzero(amax >= 2.0)[0]:
            rk = uq[(uq // N == n) & (c == amax[n])] % N
            morig = (rk + h * H) % N              # back to original node ids
            duprows.append((int(n), amax[n], morig))
        # rows with no outgoing edges -> exact 0 (device yields NaN there)
        zerorows = np.nonzero(amax == 0.0)[0]
        # nodesT rotated so this core's rows are 0..511
        nrot = np.concatenate([nodes[b, h * H:], nodes[b, :h * H]], axis=0) \
            if h else nodes[b]
        nT_dev = np.ascontiguousarray(
            nrot.T.reshape(NC_F, P, N).transpose(1, 0, 2).astype(bf))
        maps.append({
            "nT": nT_dev,
            "wsb": w_dev,
            "wbc": wb_dev,
            "adjT": adjT_dev,
        })
        fixups.append((duprows, zerorows))
    return maps, fixups


def kernel(nodes, Cmat, Nmat, mask, w, attention, _trace=False, _tmpdir=None):
    from concourse.bass_utils import run_bass_kernel_spmd

    nc = _get_compiled()
    maps, fixups = _in_maps(nodes, Cmat, Nmat, w, attention)
    res = run_bass_kernel_spmd(nc, maps, list(range(N_CORES)),
                               trace=_trace, tmpdir=_tmpdir)
    full = np.empty((B, N, D), dtype=np.float32)
    for core in range(N_CORES):
        b, h = divmod(core, 2)
        o = np.asarray(res.results[core]["out"], dtype=np.float32)
        full[b, h * H:(h + 1) * H, :] = o.transpose(1, 0, 2).reshape(H, D)
    # duplicate-edge rows: reference fp32 collapses them to equal weights
    w32 = np.asarray(w, dtype=np.float32)
    nodes32 = np.asarray(nodes, dtype=np.float32)
    for core in range(N_CORES):
        b, h = divmod(core, 2)
        duprows, zerorows = fixups[core]
        for n, am, morig in duprows:
            row = (am / len(morig)) * (nodes32[b, morig].sum(0) @ w32)
            full[b, h * H + n] = np.where(row > 0, row, ALPHA * row)
        for n in zerorows:
            full[b, h * H + n] = 0.0
    if _trace:
        return full, res
    return full


if __name__ == "__main__":
    rng = np.random.default_rng(0)
    src = rng.integers(0, N, (B, E))
    dst = rng.integers(0, N, (B, E))
    Cm = np.eye(N, dtype=np.float32)[src]
    Nm = np.eye(N, dtype=np.float32)[dst]
    nodes = rng.standard_normal((B, N, F)).astype(np.float32)
    w = (rng.standard_normal((F, D)) * 0.05).astype(np.float32)
    att = (rng.standard_normal((2 * D, 1)) * 0.05).astype(np.float32)
    mask = np.ones((B, N, N), dtype=bool)
    got = kernel(nodes, Cm, Nm, mask, w, att)
    print("kernel ran, output shape", got.shape)


# revision 11
# speedup vs baseline: 1.0686x; 1.0686x over previous
"""Trainium2 Bass kernel for nn_AttGraphConvLayer (v2, transposed-E).

Reference computation (per batch b):
    z   = nodes @ w                          [N, D]
    att = leaky_relu(concat(z1, z2) @ attention)      per edge
    scores = (Cmat^T * att^T) @ Nmat         [N, N]
    adj    = Cmat^T @ Nmat                   [N, N]
    logits = scores + (1 - adj) * (-1e9)
    out = leaky_relu(softmax(logits, -1) * adj @ z)   [N, D]

Identities (Cmat/Nmat one-hot):
  * att_e = leaky(u[src] + v[dst]) with u = z @ a_top, v = z @ a_bot.
  * For rows whose max edge multiplicity is 1 (all but ~18/core), the
    reference reduces EXACTLY to out[n] = leaky(sum_m A[n,m] z[m] / Z_n)
    with A = adj .* exp(leaky(u[n]+v[m])), Z_n = sum_m A[n,m]; the exp
    row-max shift cancels between numerator and denominator in fp32.
  * Rows with multiplicity >= 2 collapse (reference fp32 artifact) to
    equal weights over the max-multiplicity edges -> host-computed.
    Rows with no edges are exactly 0 -> host-set (device yields NaN
    there via 1/Z=inf, overwritten).

v2 layout trick: the attention matrix is produced TRANSPOSED,
  ET[m, n] = adjT[m, n] * exp(leaky(v[m] + u[n]))
(partition = m = source-of-message axis, free = n = own row axis), which
is directly the lhsT of the output matmul out = ET^T @ z. This removes
the DMA transposes, turns the mask multiply into a bf16 2x-mode DVE op,
and lets output matmuls consume ET chunk-by-chunk as ACT produces them.
Z row sums come from PE ones-matmuls into a [1, H] psum accumulated
alongside; rcpf = 1/Z is transposed back via 4 tiny PE transposes.

Sharding: 8 cores = 4 batches x 2 row-halves (partition by source node).
The host rotates the node axis per core so its 512 output rows are the
first 512 node columns.
"""

import sys

for _p in ("/opt/trn_rl_repo", "/root/.axon_site/_ro/trn_rl_repo"):
    if _p not in sys.path:
        sys.path.insert(0, _p)

import numpy as np

B, E, N, F, D = 4, 8192, 1024, 512, 512
H = N // 2          # rows per core
P = 128
ALPHA = 0.2
N_CORES = 8
NC_F = F // P       # 4 feature chunks
NC_N = N // P       # 8 node (m) chunks
NC_H = H // P       # 4 own-row chunks

_compiled = None


def _build():
    import concourse.bacc as bacc
    import concourse.tile as tile
    import concourse.mybir as mybir

    dt = mybir.dt
    f32 = dt.float32
    bf16 = dt.bfloat16
    fp8 = dt.float8e4
    Act = mybir.ActivationFunctionType

    nc = bacc.Bacc("TRN2", target_bir_lowering=False, debug=False,
                   num_devices=N_CORES)

    # dram tensors are laid out host-side exactly as SBUF wants them
    nT = nc.dram_tensor("nT", [P, 2, NC_F, H], bf16,
                        kind="ExternalInput").ap()
    wsb = nc.dram_tensor("wsb", [P, NC_F, D], bf16, kind="ExternalInput").ap()
    wbc = nc.dram_tensor("wbc", [P, NC_F, 2], bf16, kind="ExternalInput").ap()
    adjT = nc.dram_tensor("adjT", [P, NC_N, H], fp8, kind="ExternalInput").ap()
    out = nc.dram_tensor("out", [P, NC_H, D], bf16, kind="ExternalOutput").ap()

    with tile.TileContext(nc) as tc:
        with tc.tile_pool(name="singles", bufs=1) as singles:
            nT_sb = singles.tile([P, 2, NC_F, H], bf16, name="nT_sb")
            w_sb = singles.tile([P, NC_F, D], bf16, name="w_sb")
            wb_sb = singles.tile([P, NC_F, 2], bf16, name="wb_sb")
            adjT_sb = singles.tile([P, NC_N, H], bf16, name="adjT_sb")
            z_sb = singles.tile([P, NC_N, D], bf16, name="z_sb")
            ET_sb = singles.tile([P, NC_N, H], bf16, name="ET_sb")
            U_bc = singles.tile([P, H], f32, name="U_bc")
            uv_sb = singles.tile([2, H], f32, name="uv_sb")
            vb_sb = singles.tile([2, H], f32, name="vb_sb")
            u_row = singles.tile([1, H], f32, name="u_row")
            v_col = singles.tile([P, NC_N], f32, name="v_col")
            ones_c = singles.tile([P, 1], bf16, name="ones_c")
            ones_r = singles.tile([1, P], f32, name="ones_r")
            ident1 = singles.tile([1, 1], f32, name="ident1")
            dumm = singles.tile([1, 1], f32, name="dumm")
            Z_sb = singles.tile([1, H], f32, name="Z_sb")
            Zc_sb = singles.tile([P, NC_H], f32, name="Zc_sb")
            rcpf = singles.tile([P, NC_H], f32, name="rcpf")

            warm_sb = singles.tile([P, D], bf16, name="warm_sb")
            nc.vector.memset(ones_c, 1.0)
            nc.gpsimd.memset(ones_r, 1.0)
            nc.gpsimd.memset(ident1, 1.0)
            nc.vector.memset(warm_sb, 1.0)
            # preload the exp ACT table during the DMA phase
            nc.scalar.activation(dumm, ident1, Act.Exp)

            # ---- input DMAs ----
            # tiny first; then nT column-halves (contiguous in dram) on the
            # sync ring -- matvecs + z chunks 0-3 unlock after the first
            # half -- with w in parallel on the scalar ring; adjT
            # fp8->bf16 cast-loads on the gpsimd SWDGE ring, one per
            # m-chunk so ET mults unlock early.
            nc.scalar.dma_start(out=wb_sb, in_=wbc)
            nc.sync.dma_start(out=nT_sb[:, 0], in_=nT[:, 0])
            nc.scalar.dma_start(out=w_sb, in_=wsb)
            nc.sync.dma_start(out=nT_sb[:, 1], in_=nT[:, 1])
            for cm in range(NC_N):
                nc.gpsimd.dma_start(out=adjT_sb[:, cm, :], in_=adjT[:, cm, :])

            # dummy matmuls on memset data warm the PE HAM clock-gate
            # while the real inputs stream in (M=1 matvecs alone leave the
            # activity monitor cold -- only full-width work counts)
            warm_ps = tc.alloc_tile_pool(name="warm_ps", bufs=1, space="PSUM")
            wp = warm_ps.tile([P, D], f32, name="wp", tag="wp")
            for i in range(12):
                nc.tensor.matmul(wp, lhsT=warm_sb[:, 0:P], rhs=warm_sb,
                                 start=True, stop=True)
            warm_ps.release()

            with tc.tile_pool(name="vu_ps", bufs=1, space="PSUM") as vu_ps, \
                 tc.tile_pool(name="usc", bufs=1, space="PSUM") as usc:
                # u/v matvecs: three M=1 accumulations so every row the PE
                # later reads sits on partition 0 (base-partition rule)
                v0p = vu_ps.tile([1, H], f32, name="v0p", tag="v0p")
                v1p = vu_ps.tile([1, H], f32, name="v1p", tag="v1p")
                u_p = vu_ps.tile([1, H], f32, name="u_p", tag="u_p")
                for cf in range(NC_F):
                    nc.tensor.matmul(v0p, lhsT=wb_sb[:, cf, 0:1],
                                     rhs=nT_sb[:, 0, cf, :],
                                     start=(cf == 0), stop=(cf == NC_F - 1))
                for cf in range(NC_F):
                    nc.tensor.matmul(u_p, lhsT=wb_sb[:, cf, 1:2],
                                     rhs=nT_sb[:, 0, cf, :],
                                     start=(cf == 0), stop=(cf == NC_F - 1))
                nc.vector.tensor_copy(uv_sb[0:1, :], v0p)
                nc.vector.tensor_copy(u_row, u_p)
                for cf in range(NC_F):
                    nc.tensor.matmul(v1p, lhsT=wb_sb[:, cf, 0:1],
                                     rhs=nT_sb[:, 1, cf, :],
                                     start=(cf == 0), stop=(cf == NC_F - 1))
                nc.vector.tensor_copy(vb_sb[0:1, :], v1p)

                # v as per-partition columns (8 tiny PE transposes), and
                # u broadcast along partitions (rank-1 ones matmul)
                vc_ps = usc.tile([P, NC_N], f32, name="vc_ps", tag="vc")
                for q in range(NC_H):
                    nc.tensor.transpose(vc_ps[:, q:q + 1],
                                        uv_sb[0:1, q * P:(q + 1) * P], ident1)
                nc.vector.tensor_copy(v_col[:, 0:NC_H], vc_ps[:, 0:NC_H])
                U_ps = usc.tile([P, H], f32, name="U_ps", tag="ups")
                nc.tensor.matmul(U_ps, lhsT=ones_r, rhs=u_row,
                                 start=True, stop=True)
                nc.vector.tensor_copy(U_bc, U_ps)
                for q in range(NC_H, NC_N):
                    nc.tensor.transpose(
                        vc_ps[:, q:q + 1],
                        vb_sb[0:1, (q - NC_H) * P:(q - NC_H + 1) * P], ident1)
                nc.vector.tensor_copy(v_col[:, NC_H:NC_N], vc_ps[:, NC_H:NC_N])

            # ---- z = nodes @ w, chunk cn completes asap; cast to bf16 ----
            o_ps = tc.alloc_tile_pool(name="o_ps", bufs=1, space="PSUM")
            ops = [o_ps.tile([P, D], f32, name=f"op_{r}", tag=f"op_{r}")
                   for r in range(NC_H)]
            Z_ps = o_ps.tile([1, H], f32, name="Z_ps", tag="zps")
            z_ps = tc.alloc_tile_pool(name="z_ps", bufs=3, space="PSUM")
            for cn in range(NC_N):
                jm, q = divmod(cn, NC_H)
                zp = z_ps.tile([P, D], f32, name=f"zp_{cn}", tag="zp")
                for cf in range(NC_F):
                    nc.tensor.matmul(zp,
                                     lhsT=nT_sb[:, jm, cf,
                                                q * P:(q + 1) * P],
                                     rhs=w_sb[:, cf, :],
                                     start=(cf == 0), stop=(cf == NC_F - 1))
                nc.vector.tensor_copy(z_sb[:, cn, :], zp)

            # ---- per m-chunk: ET = adjT .* exp(leaky(v + u)), then the
            # output matmuls consume each chunk as it lands ----
            with tc.tile_pool(name="pscr", bufs=3) as pscr, \
                 tc.tile_pool(name="escr", bufs=3) as escr:
                for cm in range(NC_N):
                    pt = pscr.tile([P, H], f32, name=f"pt_{cm}", tag="pt")
                    nc.scalar.activation(pt, U_bc, Act.Prelu,
                                         bias=v_col[:, cm:cm + 1], scale=1.0,
                                         alpha=ALPHA)
                    et = escr.tile([P, H], bf16, name=f"et_{cm}", tag="et")
                    nc.scalar.activation(et, pt, Act.Exp)
                    nc.vector.tensor_mul(ET_sb[:, cm, :], et,
                                         adjT_sb[:, cm, :])
                    for r in range(NC_H):
                        nc.tensor.matmul(
                            ops[r], lhsT=ET_sb[:, cm, r * P:(r + 1) * P],
                            rhs=z_sb[:, cm, :],
                            start=(cm == 0), stop=(cm == NC_N - 1))
                    nc.tensor.matmul(Z_ps, lhsT=ones_c, rhs=ET_sb[:, cm, :],
                                     start=(cm == 0), stop=(cm == NC_N - 1))
            z_ps.release()

            # ---- rcpf = 1/Z as per-partition columns (transpose FIRST:
            # reciprocal on [128,4] runs 128 lanes; on [1,512] it is a
            # single-lane iterative divide, ~3.3us) ----
            rc_ps = tc.alloc_tile_pool(name="rc_ps", bufs=1, space="PSUM")
            nc.vector.tensor_copy(Z_sb, Z_ps)
            rcp_t = rc_ps.tile([P, NC_H], f32, name="rcp_t", tag="rct")
            for q in range(NC_H):
                nc.tensor.transpose(rcp_t[:, q:q + 1],
                                    Z_sb[:, q * P:(q + 1) * P], ident1)
            nc.vector.tensor_copy(Zc_sb, rcp_t)
            nc.vector.reciprocal(rcpf, Zc_sb)

            # ---- out = leaky(rcpf * psum), store ----
            with tc.tile_pool(name="oscr", bufs=4) as oscr:
                for r in range(NC_H):
                    o_l = oscr.tile([P, D], bf16, name=f"ol_{r}", tag="ol")
                    nc.scalar.activation(o_l, ops[r], Act.Prelu, bias=0.0,
                                         scale=rcpf[:, r:r + 1], alpha=ALPHA)
                    eng = (nc.sync, nc.scalar, nc.sync, nc.scalar)[r]
                    eng.dma_start(out=out[:, r, :], in_=o_l)
            rc_ps.release()
            o_ps.release()

    nc.compile()
    return nc


def _get_compiled():
    global _compiled
    if _compiled is None:
        _compiled = _build()
    return _compiled


def _in_maps(nodes, Cmat, Nmat, w, attention):
    import ml_dtypes
    f8 = ml_dtypes.float8_e4m3
    bf = ml_dtypes.bfloat16
    nodes = np.asarray(nodes, dtype=np.float32)
    w = np.ascontiguousarray(np.asarray(w, dtype=np.float32))
    attention = np.asarray(attention, dtype=np.float32)
    fixups = []
    wa = w @ attention[:D, 0]                     # [F] = w @ a_top
    wb = w @ attention[D:, 0]                     # [F] = w @ a_bot
    wb_dev = np.ascontiguousarray(
        np.stack([wb.reshape(NC_F, P).T, wa.reshape(NC_F, P).T],
                 axis=2).astype(bf))              # [P, NC_F, 2]: col0=v, col1=u
    w_dev = np.ascontiguousarray(
        w.reshape(NC_F, P, D).transpose(1, 0, 2).astype(bf))
    maps = []
    for core in range(N_CORES):
        b, h = divmod(core, 2)
        src = Cmat[b].argmax(axis=1)
        dst = Nmat[b].argmax(axis=1)
        srcp = (src - h * H) % N                  # rotated node ids
        dstp = (dst - h * H) % N
        own = srcp < H
        sp, dp = srcp[own], dstp[own]
        # transposed adjacency counts: adjT[m, n] = #edges (src=n, dst=m)
        adjT = np.zeros((N, H), np.float32)
        np.add.at(adjT, (dp, sp), 1.0)
        adjT_dev = np.ascontiguousarray(
            adjT.reshape(NC_N, P, H).transpose(1, 0, 2).astype(f8))
        # per-own-row max edge multiplicity (host stat for fixups)
        key = sp.astype(np.int64) * N + dp
        uq, c = np.unique(key, return_counts=True)
        amax = np.zeros(H, np.float32)
        np.maximum.at(amax, (uq // N).astype(np.int64), c.astype(np.float32))
        duprows = []
        for n in np.nonzero(amax >= 2.0)[0]:
            rk = uq[(uq // N == n) & (c == amax[n])] % N
            morig = (rk + h * H) % N              # back to original node ids
            duprows.append((int(n), amax[n], morig))
        # rows with no outgoing edges -> exact 0 (device yields NaN there)
        zerorows = np.nonzero(amax == 0.0)[0]
        # nodesT rotated so this core's rows are 0..511
        nrot = np.concatenate([nodes[b, h * H:], nodes[b, :h * H]], axis=0) \
            if h else nodes[b]
        # [P, 2, NC_F, H]: contiguous column-halves for fast DMA
        nT_dev = np.ascontiguousarray(
            nrot.T.reshape(NC_F, P, 2, H).transpose(1, 2, 0, 3).astype(bf))
        maps.append({
            "nT": nT_dev,
            "wsb": w_dev,
            "wbc": wb_dev,
            "adjT": adjT_dev,
        })
        fixups.append((duprows, zerorows))
    return maps, fixups


def kernel(nodes, Cmat, Nmat, mask, w, attention, _trace=False, _tmpdir=None):
    from concourse.bass_utils import run_bass_kernel_spmd

    nc = _get_compiled()
    maps, fixups = _in_maps(nodes, Cmat, Nmat, w, attention)
    res = run_bass_kernel_spmd(nc, maps, list(range(N_CORES)),
                               trace=_trace, tmpdir=_tmpdir)
    full = np.empty((B, N, D), dtype=np.float32)
    for core in range(N_CORES):
        b, h = divmod(core, 2)
        o = np.asarray(res.results[core]["out"], dtype=np.float32)
        full[b, h * H:(h + 1) * H, :] = o.transpose(1, 0, 2).reshape(H, D)
    # duplicate-edge rows: reference fp32 collapses them to equal weights
    w32 = np.asarray(w, dtype=np.float32)
    nodes32 = np.asarray(nodes, dtype=np.float32)
    for core in range(N_CORES):
        b, h = divmod(core, 2)
        duprows, zerorows = fixups[core]
        for n, am, morig in duprows:
            row = (am / len(morig)) * (nodes32[b, morig].sum(0) @ w32)
            full[b, h * H + n] = np.where(row > 0, row, ALPHA * row)
        for n in zerorows:
            full[b, h * H + n] = 0.0
    if _trace:
        return full, res
    return full


if __name__ == "__main__":
    rng = np.random.default_rng(0)
    src = rng.integers(0, N, (B, E))
    dst = rng.integers(0, N, (B, E))
    Cm = np.eye(N, dtype=np.float32)[src]
    Nm = np.eye(N, dtype=np.float32)[dst]
    nodes = rng.standard_normal((B, N, F)).astype(np.float32)
    w = (rng.standard_normal((F, D)) * 0.05).astype(np.float32)
    att = (rng.standard_normal((2 * D, 1)) * 0.05).astype(np.float32)
    mask = np.ones((B, N, N), dtype=bool)
    got = kernel(nodes, Cm, Nm, mask, w, att)
    print("kernel ran, output shape", got.shape)
